# revision 18
# baseline (speedup 1.0000x reference)
"""Trainium2 kernel for nn_KerasDense_32263794328408.

y = relu(x @ M + b), where M is a 4096x4096 TT-matrix (cores of shape
[r_{k-1}, 8, 8, r_k], ranks [1,8,8,8,1]).

Strategy: the TT cores are tiny (<17 KB each); materialize the dense
M = 4096x4096 on the host (cheap, ~270 MFLOP) and run the dense
y = relu(x @ M + b) as a near-roofline GEMM on 8 NeuronCores.

Sharding: 2D grid, 4 batch groups x 2 output-column groups.
Per core: x-shard [1024, 4096] (shipped transposed as xT [4096, 1024]),
W column-half [4096, 2048] and bias half, producing y [1024, 2048].

On-chip: x-stationary matmuls with fp32r (FP22 truncated fp32, full PE
rate at N=512). lhsT = xT tile [128k x 128b], rhs = W slab [128k x 512o],
PSUM accumulation over the 32 k-tiles plus a K=1 matmul adding the bias,
then a fused relu copy (ScalarE) and contiguous DMA out.
"""

import sys

if "/opt/trn_rl_repo" not in sys.path:
    sys.path.insert(0, "/opt/trn_rl_repo")

import numpy as np

import concourse.bacc as bacc
import concourse.bass as bass
import concourse.mybir as mybir
import concourse.tile as tile
from concourse.bass_utils import run_bass_kernel_spmd

F32 = mybir.dt.float32
F32R = mybir.dt.float32r

B_FULL = 4096  # batch
F_FULL = 4096  # input features
O_FULL = 4096  # output features

BG = 4  # batch groups
OG = 2  # output-column groups
N_CORES = BG * OG

B_L = B_FULL // BG   # 1024 batch rows per core
O_L = O_FULL // OG   # 2048 output cols per core
KT = F_FULL // 128   # 32 contraction tiles
OC = O_L // 512      # 4 output chunks of 512 per core
BT = B_L // 128      # 8 batch tiles of 128 per core

_CACHE: dict = {}


def _build_module() -> bass.Bass:
    nc = bacc.Bacc(None, target_bir_lowering=False)

    xT = nc.declare_dram_parameter("xT", [F_FULL, B_L], F32R, isOutput=False)
    w = nc.declare_dram_parameter("w", [F_FULL, O_L], F32R, isOutput=False)
    bvec = nc.declare_dram_parameter("bvec", [1, O_L], F32R, isOutput=False)
    ones = nc.declare_dram_parameter("ones", [1, 128], F32R, isOutput=False)
    y = nc.declare_dram_parameter("y", [B_L, O_L], F32, isOutput=True)

    KQ = 4           # k-tiles fetched per W DMA (quad slabs of [128, KQ*512])
    NQ = KT // KQ    # 8 quad fetches per oc

    with tile.TileContext(nc) as tc:
        with (
            tc.tile_pool(name="xt", bufs=1) as xt_pool,
            tc.tile_pool(name="wsl", bufs=4) as w_pool,
            tc.tile_pool(name="yst", bufs=3) as y_pool,
            tc.tile_pool(name="cst", bufs=1) as c_pool,
            tc.tile_pool(name="acc", bufs=8, space="PSUM") as psum_pool,
        ):
            # xT resident in SBUF as one tile per k-tile so consumers only
            # wait on their own 512 KB load. Loads go on the ACT HWDGE ring.
            xts = []
            for kt in range(KT):
                t = xt_pool.tile([128, B_L], F32R, tag=f"xt{kt}", name=f"xt{kt}")
                nc.scalar.dma_start(out=t[:], in_=xT[kt * 128 : (kt + 1) * 128, :])
                xts.append(t)

            bias_sb = c_pool.tile([1, O_L], F32R, tag="bias")
            nc.scalar.dma_start(out=bias_sb[:], in_=bvec[:])
            ones_sb = c_pool.tile([1, 128], F32R, tag="ones")
            nc.scalar.dma_start(out=ones_sb[:], in_=ones[:])

            for oc in range(OC):
                accs = []
                for bt in range(BT):
                    accs.append(
                        psum_pool.tile(
                            [128, 512], F32, tag="acc", name=f"acc_{oc}_{bt}"
                        )
                    )
                for ktq in range(NQ):
                    if oc == 0 and ktq == 0:
                        # Head-latency fix: fetch the first 4 k-tiles as
                        # separate 256 KB slabs so kt=0 matmuls start early.
                        slabs = []
                        for k4 in range(KQ):
                            s = w_pool.tile([128, 512], F32R, tag="wsl0",
                                            name=f"w0_{k4}")
                            nc.sync.dma_start(
                                out=s[:],
                                in_=w[k4 * 128 : (k4 + 1) * 128, 0:512],
                            )
                            slabs.append(s)
                        w_slices = [s[:] for s in slabs]
                    else:
                        # One 1 MB DMA fetches 4 k-tiles of W for this oc.
                        w_sl = w_pool.tile([128, KQ * 512], F32R, tag="wsl",
                                           name=f"w_{oc}_{ktq}")
                        src = w[
                            ktq * (KQ * 128) : (ktq + 1) * (KQ * 128),
                            oc * 512 : (oc + 1) * 512,
                        ].rearrange("(k p) c -> p k c", k=KQ)
                        dst = w_sl[:].rearrange("p (k c) -> p k c", k=KQ)
                        nc.sync.dma_start(out=dst, in_=src)
                        w_slices = [
                            w_sl[:, k4 * 512 : (k4 + 1) * 512] for k4 in range(KQ)
                        ]
                    for k4 in range(KQ):
                        kt = ktq * KQ + k4
                        for bt in range(BT):
                            nc.tensor.matmul(
                                accs[bt][:],
                                xts[kt][:, bt * 128 : (bt + 1) * 128],
                                w_slices[k4],
                                start=(kt == 0),
                                stop=False,
                            )
                for bt in range(BT):
                    # += 1 (x) bias  via a K=1 matmul: closes the accumulation.
                    nc.tensor.matmul(
                        accs[bt][:],
                        ones_sb[:],
                        bias_sb[:, oc * 512 : (oc + 1) * 512],
                        start=False,
                        stop=True,
                    )
                    y_sl = y_pool.tile(
                        [128, 512], F32, tag="yst", name=f"y_{oc}_{bt}"
                    )
                    # Split the PSUM->SBUF relu drain across ACT and DVE so
                    # the per-oc drain chain (and the kernel tail) is 2x wide.
                    if bt % 2 == 0:
                        nc.scalar.activation(
                            y_sl[:], accs[bt][:], mybir.ActivationFunctionType.Relu
                        )
                    else:
                        nc.vector.tensor_scalar_max(y_sl[:], accs[bt][:], 0.0)
                    dma_eng = nc.sync if oc == OC - 1 else nc.scalar
                    dma_eng.dma_start(
                        out=y[bt * 128 : (bt + 1) * 128, oc * 512 : (oc + 1) * 512],
                        in_=y_sl[:],
                    )

    nc.finalize()
    return nc


def _materialize_w(core0, core1, core2, core3) -> np.ndarray:
    """Contract the TT cores into the dense 4096x4096 matrix M.

    M[(m1 m2 m3 m4), (n1 n2 n3 n4)] (big-endian mode order on both sides),
    matching the reference's x/y index conventions.
    """
    g1 = core0[0].astype(np.float64)            # [m1, n1, r1]
    t12 = np.einsum("mnr,rMNs->mMnNs", g1, core1.astype(np.float64))
    a12 = t12.reshape(64, 64, 8)                # [(m1 m2), (n1 n2), r2]
    g4 = core3[..., 0].astype(np.float64)       # [r3, m4, n4]
    t34 = np.einsum("rmns,sMN->rmMnN", core2.astype(np.float64), g4)
    b34 = t34.reshape(8, 64, 64)                # [r2, (m3 m4), (n3 n4)]
    w = np.einsum("mnr,rMN->mMnN", a12, b34)    # [(m12),(m34),(n12),(n34)]
    return np.ascontiguousarray(
        w.reshape(F_FULL, O_FULL), dtype=np.float32
    )


def kernel(x, core0, core1, core2, core3, b) -> np.ndarray:
    x = np.asarray(x, dtype=np.float32)
    w_full = _materialize_w(
        np.asarray(core0, dtype=np.float32),
        np.asarray(core1, dtype=np.float32),
        np.asarray(core2, dtype=np.float32),
        np.asarray(core3, dtype=np.float32),
    )
    bias = np.asarray(b, dtype=np.float32).reshape(1, O_FULL)

    if "nc" not in _CACHE:
        _CACHE["nc"] = _build_module()
    nc = _CACHE["nc"]

    # Shard: core c = (g, h); batch group g, output-column group h.
    xts = [
        np.ascontiguousarray(x[g * B_L : (g + 1) * B_L, :].T)
        for g in range(BG)
    ]
    ws = [
        np.ascontiguousarray(w_full[:, h * O_L : (h + 1) * O_L])
        for h in range(OG)
    ]
    bs = [np.ascontiguousarray(bias[:, h * O_L : (h + 1) * O_L]) for h in range(OG)]

    in_maps = []
    for c in range(N_CORES):
        g, h = divmod(c, OG)
        in_maps.append(
            {
                "xT": xts[g],
                "w": ws[h],
                "bvec": bs[h],
                "ones": np.ones((1, 128), dtype=np.float32),
            }
        )

    res = run_bass_kernel_spmd(nc, in_maps, core_ids=list(range(N_CORES)))

    y = np.empty((B_FULL, O_FULL), dtype=np.float32)
    for c in range(N_CORES):
        g, h = divmod(c, OG)
        y[g * B_L : (g + 1) * B_L, h * O_L : (h + 1) * O_L] = res.results[c]["y"]
    return y


# revision 19
# speedup vs baseline: 1.0218x; 1.0218x over previous
"""Trainium2 kernel for nn_KerasDense_32263794328408.

y = relu(x @ M + b), where M is a 4096x4096 TT-matrix (cores of shape
[r_{k-1}, 8, 8, r_k], ranks [1,8,8,8,1]).

Strategy: the TT cores are tiny (<17 KB each); materialize the dense
M = 4096x4096 on the host (cheap, ~270 MFLOP) and run the dense
y = relu(x @ M + b) as a near-roofline GEMM on 8 NeuronCores.

Sharding: 2D grid, 4 batch groups x 2 output-column groups.
Per core: x-shard [1024, 4096] (shipped transposed as xT [4096, 1024]),
W column-half [4096, 2048] and bias half, producing y [1024, 2048].

On-chip: x-stationary matmuls with fp32r (FP22 truncated fp32, full PE
rate at N=512). lhsT = xT tile [128k x 128b], rhs = W slab [128k x 512o],
PSUM accumulation over the 32 k-tiles plus a K=1 matmul adding the bias,
then a fused relu copy (ScalarE) and contiguous DMA out.
"""

import sys

if "/opt/trn_rl_repo" not in sys.path:
    sys.path.insert(0, "/opt/trn_rl_repo")

import numpy as np

import concourse.bacc as bacc
import concourse.bass as bass
import concourse.mybir as mybir
import concourse.tile as tile
from concourse.bass_utils import run_bass_kernel_spmd

F32 = mybir.dt.float32
F32R = mybir.dt.float32r

B_FULL = 4096  # batch
F_FULL = 4096  # input features
O_FULL = 4096  # output features

BG = 4  # batch groups
OG = 2  # output-column groups
N_CORES = BG * OG

B_L = B_FULL // BG   # 1024 batch rows per core
O_L = O_FULL // OG   # 2048 output cols per core
KT = F_FULL // 128   # 32 contraction tiles
OC = O_L // 512      # 4 output chunks of 512 per core
BT = B_L // 128      # 8 batch tiles of 128 per core

_CACHE: dict = {}


def _build_module() -> bass.Bass:
    nc = bacc.Bacc(None, target_bir_lowering=False)

    xT = nc.declare_dram_parameter("xT", [F_FULL, B_L], F32R, isOutput=False)
    w = nc.declare_dram_parameter("w", [F_FULL, O_L], F32R, isOutput=False)
    bvec = nc.declare_dram_parameter("bvec", [1, O_L], F32R, isOutput=False)
    ones = nc.declare_dram_parameter("ones", [1, 128], F32R, isOutput=False)
    y = nc.declare_dram_parameter("y", [B_L, O_L], F32, isOutput=True)

    KQ = 4           # k-tiles fetched per W DMA (quad slabs of [128, KQ*512])
    NQ = KT // KQ    # 8 quad fetches per oc

    with tile.TileContext(nc) as tc:
        with (
            tc.tile_pool(name="xt", bufs=1) as xt_pool,
            tc.tile_pool(name="wsl", bufs=4) as w_pool,
            tc.tile_pool(name="yst", bufs=3) as y_pool,
            tc.tile_pool(name="cst", bufs=1) as c_pool,
            tc.tile_pool(name="acc", bufs=8, space="PSUM") as psum_pool,
        ):
            # xT resident in SBUF as one tile per k-tile so consumers only
            # wait on their own 512 KB load. Loads go on the ACT HWDGE ring.
            xts = []
            for kt in range(KT):
                t = xt_pool.tile([128, B_L], F32R, tag=f"xt{kt}", name=f"xt{kt}")
                nc.scalar.dma_start(out=t[:], in_=xT[kt * 128 : (kt + 1) * 128, :])
                xts.append(t)

            bias_sb = c_pool.tile([1, O_L], F32R, tag="bias")
            nc.scalar.dma_start(out=bias_sb[:], in_=bvec[:])
            ones_sb = c_pool.tile([1, 128], F32R, tag="ones")
            nc.scalar.dma_start(out=ones_sb[:], in_=ones[:])

            for oc in range(OC):
                accs = []
                for bt in range(BT):
                    accs.append(
                        psum_pool.tile(
                            [128, 512], F32, tag="acc", name=f"acc_{oc}_{bt}"
                        )
                    )
                for ktq in range(NQ):
                    if oc == 0 and ktq == 0:
                        # Head-latency fix: fetch the first 4 k-tiles as
                        # separate 256 KB slabs so kt=0 matmuls start early.
                        slabs = []
                        for k4 in range(KQ):
                            s = w_pool.tile([128, 512], F32R, tag="wsl0",
                                            name=f"w0_{k4}")
                            nc.sync.dma_start(
                                out=s[:],
                                in_=w[k4 * 128 : (k4 + 1) * 128, 0:512],
                            )
                            slabs.append(s)
                        w_slices = [s[:] for s in slabs]
                    else:
                        # One 1 MB DMA fetches 4 k-tiles of W for this oc.
                        w_sl = w_pool.tile([128, KQ * 512], F32R, tag="wsl",
                                           name=f"w_{oc}_{ktq}")
                        src = w[
                            ktq * (KQ * 128) : (ktq + 1) * (KQ * 128),
                            oc * 512 : (oc + 1) * 512,
                        ].rearrange("(k p) c -> p k c", k=KQ)
                        dst = w_sl[:].rearrange("p (k c) -> p k c", k=KQ)
                        nc.sync.dma_start(out=dst, in_=src)
                        w_slices = [
                            w_sl[:, k4 * 512 : (k4 + 1) * 512] for k4 in range(KQ)
                        ]
                    for k4 in range(KQ):
                        kt = ktq * KQ + k4
                        for bt in range(BT):
                            nc.tensor.matmul(
                                accs[bt][:],
                                xts[kt][:, bt * 128 : (bt + 1) * 128],
                                w_slices[k4],
                                start=(kt == 0),
                                stop=False,
                            )
                for bt in range(BT):
                    # += 1 (x) bias  via a K=1 matmul: closes the accumulation.
                    nc.tensor.matmul(
                        accs[bt][:],
                        ones_sb[:],
                        bias_sb[:, oc * 512 : (oc + 1) * 512],
                        start=False,
                        stop=True,
                    )
                    y_sl = y_pool.tile(
                        [128, 512], F32, tag="yst", name=f"y_{oc}_{bt}"
                    )
                    nc.scalar.activation(
                        y_sl[:], accs[bt][:], mybir.ActivationFunctionType.Relu
                    )
                    dma_eng = nc.sync if oc == OC - 1 else nc.scalar
                    dma_eng.dma_start(
                        out=y[bt * 128 : (bt + 1) * 128, oc * 512 : (oc + 1) * 512],
                        in_=y_sl[:],
                    )

    nc.finalize()
    return nc


def _materialize_w(core0, core1, core2, core3) -> np.ndarray:
    """Contract the TT cores into the dense 4096x4096 matrix M.

    M[(m1 m2 m3 m4), (n1 n2 n3 n4)] (big-endian mode order on both sides),
    matching the reference's x/y index conventions.
    """
    g1 = core0[0].astype(np.float64)            # [m1, n1, r1]
    t12 = np.einsum("mnr,rMNs->mMnNs", g1, core1.astype(np.float64))
    a12 = t12.reshape(64, 64, 8)                # [(m1 m2), (n1 n2), r2]
    g4 = core3[..., 0].astype(np.float64)       # [r3, m4, n4]
    t34 = np.einsum("rmns,sMN->rmMnN", core2.astype(np.float64), g4)
    b34 = t34.reshape(8, 64, 64)                # [r2, (m3 m4), (n3 n4)]
    w = np.einsum("mnr,rMN->mMnN", a12, b34)    # [(m12),(m34),(n12),(n34)]
    return np.ascontiguousarray(
        w.reshape(F_FULL, O_FULL), dtype=np.float32
    )


def kernel(x, core0, core1, core2, core3, b) -> np.ndarray:
    x = np.asarray(x, dtype=np.float32)
    w_full = _materialize_w(
        np.asarray(core0, dtype=np.float32),
        np.asarray(core1, dtype=np.float32),
        np.asarray(core2, dtype=np.float32),
        np.asarray(core3, dtype=np.float32),
    )
    bias = np.asarray(b, dtype=np.float32).reshape(1, O_FULL)

    if "nc" not in _CACHE:
        _CACHE["nc"] = _build_module()
    nc = _CACHE["nc"]

    # Shard: core c = (g, h); batch group g, output-column group h.
    xts = [
        np.ascontiguousarray(x[g * B_L : (g + 1) * B_L, :].T)
        for g in range(BG)
    ]
    ws = [
        np.ascontiguousarray(w_full[:, h * O_L : (h + 1) * O_L])
        for h in range(OG)
    ]
    bs = [np.ascontiguousarray(bias[:, h * O_L : (h + 1) * O_L]) for h in range(OG)]

    in_maps = []
    for c in range(N_CORES):
        g, h = divmod(c, OG)
        in_maps.append(
            {
                "xT": xts[g],
                "w": ws[h],
                "bvec": bs[h],
                "ones": np.ones((1, 128), dtype=np.float32),
            }
        )

    res = run_bass_kernel_spmd(nc, in_maps, core_ids=list(range(N_CORES)))

    y = np.empty((B_FULL, O_FULL), dtype=np.float32)
    for c in range(N_CORES):
        g, h = divmod(c, OG)
        y[g * B_L : (g + 1) * B_L, h * O_L : (h + 1) * O_L] = res.results[c]["y"]
    return y
